# revision 6
# baseline (speedup 1.0000x reference)
"""Trainium2 Bass kernel for AttentionPETL (B=16, N=1024, C=768, H=12).

Strategy: data-parallel over batch across 8 NeuronCores (2 batches/core).
Per core, per batch:
  - qkv projection as PE matmuls (float32r), operands pre-transposed on host
    (xT = x^T per batch, wT = qkv_w^T with the q-part pre-scaled by D^-0.5).
    Q,K stored [d, n] (bf16), V stored [n, d] (bf16).
  - Per head-pair (row/col 32x32-tile pairing on the PE array):
      S^T = K^T Q  -> exp on ScalarE -> P^T (bf16, unnormalized)
      O'^T = sum_m V P^T   (PSUM accumulation over 8 m-tiles)
      row-sums of P^T via ones-matmul  -> reciprocal -> O^T = O'^T * inv (DVE)
      S = Q^T K -> exp (accum_out gives softmax denominators) -> A = P * inv
      A tiles DMA straight out as attn[b,h,n,:] rows.
  - out = O^T^T @ proj_w^T + proj_b via PE (bf16), written as [n, o] rows.
Blur-head masking == zeroing the target_cls column of Q for listed heads
(entire logit row becomes 0 -> softmax uniform), baked in as tiny memsets.
"""

import os
from contextlib import ExitStack

import numpy as np
import ml_dtypes

import concourse.bass as bass
import concourse.bacc as bacc
import concourse.tile as tile
import concourse.mybir as mybir
import concourse.bass_utils as bass_utils

B, N, C = 16, 1024, 768
H, D = 12, 64
NCORES = 8
BL = B // NCORES          # batches per core
CT = C // 128             # 6 c-tiles
NT = N // 128             # 8 n-tiles
PAIRS = H // 2            # 6 head pairs

F32 = mybir.dt.float32
F32R = mybir.dt.float32r
BF16 = mybir.dt.bfloat16
BF16_NP = ml_dtypes.bfloat16

# dtype for the qkv projection operands: float32r = full-rate PE fp32
QKV_DT = F32R

LAST_RESULTS = None       # stash of BassKernelResults for test harness


def _build(nc: bass.Bass, blur_heads, tcls: int):
    xT = nc.dram_tensor("xT", [BL, C, N], QKV_DT, kind="ExternalInput").ap()
    wT = nc.dram_tensor("wT", [C, 3 * C], QKV_DT, kind="ExternalInput").ap()
    bias_qk = nc.dram_tensor("bias_qk", [128, 2 * CT], F32, kind="ExternalInput").ap()
    b_vc = nc.dram_tensor("b_vc", [128, C], F32, kind="ExternalInput").ap()
    pwT = nc.dram_tensor("pwT", [C, C], BF16, kind="ExternalInput").ap()
    b_pc = nc.dram_tensor("b_pc", [128, C], F32, kind="ExternalInput").ap()
    out_l = nc.dram_tensor("out_l", [BL, N, C], F32, kind="ExternalOutput").ap()
    attn_l = nc.dram_tensor("attn_l", [BL, H, N, N], F32, kind="ExternalOutput").ap()

    with tile.TileContext(nc) as tc, ExitStack() as ctx:
        const = ctx.enter_context(tc.tile_pool(name="const", bufs=1))
        xt_pool = ctx.enter_context(tc.tile_pool(name="xt", bufs=1))
        qk_pool = ctx.enter_context(tc.tile_pool(name="qk", bufs=1))
        v_pool = ctx.enter_context(tc.tile_pool(name="v", bufs=1))
        o_pool = ctx.enter_context(tc.tile_pool(name="o", bufs=1))
        pt_pool = ctx.enter_context(tc.tile_pool(name="pt", bufs=2))
        p_pool = ctx.enter_context(tc.tile_pool(name="p", bufs=2))
        a_pool = ctx.enter_context(tc.tile_pool(name="a", bufs=2))
        inv_pool = ctx.enter_context(tc.tile_pool(name="invbc", bufs=2))
        st_pool = ctx.enter_context(tc.tile_pool(name="stats", bufs=4))
        po_pool = ctx.enter_context(tc.tile_pool(name="po", bufs=2))
        ps = ctx.enter_context(tc.tile_pool(name="ps", bufs=4, space="PSUM"))

        # ---- constants ----
        wt_sb = []
        for c in range(CT):
            w = const.tile([128, 3 * C], QKV_DT, name=f"wt{c}", tag=f"wt{c}")
            nc.sync.dma_start(w[:], wT[c * 128:(c + 1) * 128, :])
            wt_sb.append(w)
        pwt_sb = []
        for c in range(CT):
            w = const.tile([128, C], BF16, name=f"pwt{c}", tag=f"pwt{c}")
            nc.sync.dma_start(w[:], pwT[c * 128:(c + 1) * 128, :])
            pwt_sb.append(w)
        bqk_sb = const.tile([128, 2 * CT], F32, name="bqk", tag="bqk")
        nc.sync.dma_start(bqk_sb[:], bias_qk[:, :])
        bvc_sb = const.tile([128, C], F32, name="bvc", tag="bvc")
        nc.sync.dma_start(bvc_sb[:], b_vc[:, :])
        bpc_sb = const.tile([128, C], F32, name="bpc", tag="bpc")
        nc.sync.dma_start(bpc_sb[:], b_pc[:, :])
        ones_sb = const.tile([128, 64], BF16, name="ones", tag="ones")
        nc.vector.memset(ones_sb[:], 1.0)

        for b in range(BL):
            # ---- load x^T ----
            xt_sb = []
            for c in range(CT):
                t = xt_pool.tile([128, N], QKV_DT, name=f"xt{c}", tag=f"xt{c}")
                nc.sync.dma_start(t[:], xT[b, c * 128:(c + 1) * 128, :])
                xt_sb.append(t)

            # ---- qkv projection: Q,K in [o, n] layout ----
            qk_sb = []
            for t in range(2 * CT):
                pst = ps.tile([128, N], F32, name=f"qkps{t}", tag="ps")
                for j in range(2):
                    for c in range(CT):
                        nc.tensor.matmul(
                            pst[:, j * 512:(j + 1) * 512],
                            lhsT=wt_sb[c][:, t * 128:(t + 1) * 128],
                            rhs=xt_sb[c][:, j * 512:(j + 1) * 512],
                            start=(c == 0), stop=(c == CT - 1),
                        )
                qkt = qk_pool.tile([128, N], BF16, name=f"qk{t}", tag=f"qk{t}")
                nc.vector.tensor_scalar_add(qkt[:], pst[:], bqk_sb[:, t:t + 1])
                qk_sb.append(qkt)

            # blur-head masking: zero target column of Q for listed heads
            for hh in sorted(set(blur_heads)):
                t, rr = hh // 2, 64 * (hh % 2)
                nc.vector.memset(qk_sb[t][rr:rr + 64, tcls:tcls + 1], 0.0)

            # ---- V in [n, o] layout ----
            v_sb = []
            for nt in range(NT):
                pst = ps.tile([128, C], F32, name=f"vps{nt}", tag="ps")
                for (lo, hi) in ((0, 512), (512, 768)):
                    for c in range(CT):
                        nc.tensor.matmul(
                            pst[:, lo:hi],
                            lhsT=xt_sb[c][:, nt * 128:(nt + 1) * 128],
                            rhs=wt_sb[c][:, 2 * C + lo:2 * C + hi],
                            start=(c == 0), stop=(c == CT - 1),
                        )
                vt = v_pool.tile([128, C], BF16, name=f"v{nt}", tag=f"v{nt}")
                nc.vector.tensor_add(vt[:], pst[:], bvc_sb[:])
                v_sb.append(vt)

            # ---- attention, one head pair at a time ----
            o_sb = []
            for pr in range(PAIRS):
                h0, h1 = 2 * pr, 2 * pr + 1
                q_t, k_t = qk_sb[pr], qk_sb[CT + pr]

                o_ps = ps.tile([128, N], F32, name=f"ops{pr}", tag="ps")
                sums_ps = ps.tile([128, N], F32, name=f"sums{pr}", tag="ps")
                for mt in range(NT):
                    st0 = ps.tile([128, N], F32, name="st0", tag="ps")
                    st1 = ps.tile([128, N], F32, name="st1", tag="ps")
                    for j in range(2):
                        sl = slice(j * 512, (j + 1) * 512)
                        nc.tensor.matmul(
                            st0[:, sl],
                            lhsT=k_t[0:64, mt * 128:(mt + 1) * 128],
                            rhs=q_t[0:64, sl], start=True, stop=True)
                        nc.tensor.matmul(
                            st1[:, sl],
                            lhsT=k_t[64:128, mt * 128:(mt + 1) * 128],
                            rhs=q_t[64:128, sl], start=True, stop=True)
                    pt0 = pt_pool.tile([128, N], BF16, name="pt0", tag="pt0")
                    pt1 = pt_pool.tile([128, N], BF16, name="pt1", tag="pt1")
                    nc.scalar.activation(pt0[:], st0[:],
                                         mybir.ActivationFunctionType.Exp)
                    nc.scalar.activation(pt1[:], st1[:],
                                         mybir.ActivationFunctionType.Exp)
                    first, last = (mt == 0), (mt == NT - 1)
                    for j in range(2):
                        sl = slice(j * 512, (j + 1) * 512)
                        nc.tensor.matmul(
                            o_ps[0:64, sl],
                            lhsT=v_sb[mt][:, h0 * 64:(h0 + 1) * 64],
                            rhs=pt0[:, sl], start=first, stop=last)
                        nc.tensor.matmul(
                            o_ps[64:128, sl],
                            lhsT=v_sb[mt][:, h1 * 64:(h1 + 1) * 64],
                            rhs=pt1[:, sl], start=first, stop=last)
                        nc.tensor.matmul(
                            sums_ps[0:64, sl],
                            lhsT=ones_sb[:, 0:64],
                            rhs=pt0[:, sl], start=first, stop=last)
                        nc.tensor.matmul(
                            sums_ps[64:128, sl],
                            lhsT=ones_sb[:, 0:64],
                            rhs=pt1[:, sl], start=first, stop=last)

                inv_bc = inv_pool.tile([128, N], F32, name="invbc", tag="invbc")
                nc.vector.reciprocal(inv_bc[:], sums_ps[:])
                ot = o_pool.tile([128, N], BF16, name=f"o{pr}", tag=f"o{pr}")
                nc.vector.tensor_mul(ot[:], o_ps[:], inv_bc[:])
                o_sb.append(ot)

                # ---- S side: normalized attention rows out ----
                for nt in range(NT):
                    s0 = ps.tile([128, N], F32, name="s0", tag="ps")
                    s1 = ps.tile([128, N], F32, name="s1", tag="ps")
                    for j in range(2):
                        sl = slice(j * 512, (j + 1) * 512)
                        nc.tensor.matmul(
                            s0[:, sl],
                            lhsT=q_t[0:64, nt * 128:(nt + 1) * 128],
                            rhs=k_t[0:64, sl], start=True, stop=True)
                        nc.tensor.matmul(
                            s1[:, sl],
                            lhsT=q_t[64:128, nt * 128:(nt + 1) * 128],
                            rhs=k_t[64:128, sl], start=True, stop=True)
                    for hh, s in ((h0, s0), (h1, s1)):
                        p = p_pool.tile([128, N], F32, name="p", tag="p")
                        sums = st_pool.tile([128, 1], F32, name="sm", tag="sm")
                        nc.scalar.activation(p[:], s[:],
                                             mybir.ActivationFunctionType.Exp,
                                             accum_out=sums[:])
                        inv = st_pool.tile([128, 1], F32, name="iv", tag="iv")
                        nc.vector.reciprocal(inv[:], sums[:])
                        a = a_pool.tile([128, N], F32, name="a", tag="a")
                        nc.vector.tensor_scalar_mul(a[:], p[:], inv[:])
                        nc.sync.dma_start(
                            attn_l[b, hh, nt * 128:(nt + 1) * 128, :], a[:])

            # ---- output projection ----
            for nt in range(NT):
                pps = ps.tile([128, C], F32, name="pps", tag="ps")
                for (lo, hi) in ((0, 512), (512, 768)):
                    for c in range(CT):
                        nc.tensor.matmul(
                            pps[:, lo:hi],
                            lhsT=o_sb[c][:, nt * 128:(nt + 1) * 128],
                            rhs=pwt_sb[c][:, lo:hi],
                            start=(c == 0), stop=(c == CT - 1),
                        )
                po = po_pool.tile([128, C], F32, name="po", tag="po")
                nc.vector.tensor_add(po[:], pps[:], bpc_sb[:])
                nc.sync.dma_start(out_l[b, nt * 128:(nt + 1) * 128, :], po[:])


def kernel(x, qkv_w, qkv_b, proj_w, proj_b, blur_head_lst, target_cls,
           block_idx=None, **_unused):
    global LAST_RESULTS
    x = np.asarray(x, dtype=np.float32)
    qkv_w = np.asarray(qkv_w, dtype=np.float32)
    qkv_b = np.asarray(qkv_b, dtype=np.float32)
    proj_w = np.asarray(proj_w, dtype=np.float32)
    proj_b = np.asarray(proj_b, dtype=np.float32)
    blur_heads = [int(v) for v in np.asarray(blur_head_lst).reshape(-1)]
    tcls = int(np.asarray(target_cls))

    # ---- host-side prep ----
    scale = D ** -0.5
    W = qkv_w.copy()
    W[:C] *= scale
    bb = qkv_b.copy()
    bb[:C] *= scale
    wT = np.ascontiguousarray(W.T)                                   # [C, 3C]
    bias_qk = np.ascontiguousarray(bb[:2 * C].reshape(2 * CT, 128).T)  # [128,12]
    b_vc = np.ascontiguousarray(np.broadcast_to(bb[2 * C:], (128, C)))
    pwT = np.ascontiguousarray(proj_w.T).astype(BF16_NP)             # [C, C]
    b_pc = np.ascontiguousarray(np.broadcast_to(proj_b, (128, C)))

    nc = bacc.Bacc("TRN2", target_bir_lowering=False, debug=False,
                   enable_asserts=False, num_devices=NCORES)
    _build(nc, blur_heads, tcls)
    nc.compile()

    in_maps = []
    for cidx in range(NCORES):
        xs = x[cidx * BL:(cidx + 1) * BL]                            # [BL,N,C]
        xT = np.ascontiguousarray(xs.transpose(0, 2, 1))             # [BL,C,N]
        in_maps.append({
            "xT": xT, "wT": wT, "bias_qk": bias_qk, "b_vc": b_vc,
            "pwT": pwT, "b_pc": b_pc,
        })

    res = bass_utils.run_bass_kernel_spmd(
        nc, in_maps, core_ids=list(range(NCORES)))
    LAST_RESULTS = res

    out = np.concatenate([r["out_l"] for r in res.results], axis=0)
    attn = np.concatenate([r["attn_l"] for r in res.results], axis=0)
    return out, attn
